# revision 6
# baseline (speedup 1.0000x reference)
"""Sharded KNN retrieval kernel for Trainium2 (8 NeuronCores).

Problem: 2048 one-hot-encoded query utterances vs 100k one-hot-encoded
support utterances; top-1 nearest neighbor by squared L2, first-index
tie-breaking; output = one-hot of the winner's meanings row.

Because both sides are one-hot, squared distance reduces to
    dist(n, s) = const - 2 * match_count(n, s),   match_count in [0, 16]
so argmin(dist) = argmax(match_count) with first-index tie-break. All
arithmetic is small integers, exact in bf16/fp32, so value+index are encoded
in the matmul output itself:

  support sharded 12500 rows/core (padded to 12800 = 25 blocks of 512).
  TensorE:  psum[n_tile, s_block] = bfT.T @ (-32 * supT)   (= -32*match_count)
  VectorE:  run[nt] = min(psum + block_idx, run[nt])       (one fused
            scalar_tensor_tensor per psum tile; -32*c + b is exact and
            orders by (match_count desc, block asc) since b < 32)
  Final:    key = run*512 + j  (j = within-block offset; equals the exact
            encoding -2^14*c + s_local), then min-reduce over j.
  Host:     per-core decode (c, s_local), global lexicographic min over
            cores by (match_count desc, global_index asc), gather meanings,
            one-hot. Everything is exact integer arithmetic in fp32.
"""

import sys
import time

import numpy as np

if "/opt/trn_rl_repo" not in sys.path:
    sys.path.insert(0, "/opt/trn_rl_repo")

import ml_dtypes

VOCAB = 32
UTT_LEN = 16
K_DIM = VOCAB * UTT_LEN  # 512
N_QUERIES = 2048
S_FULL = 100000
N_CORES = 8
S_SHARD = S_FULL // N_CORES  # 12500
S_PAD = 12800
BLOCK = 512
N_BLOCKS = S_PAD // BLOCK  # 25
N_TILES = N_QUERIES // 128  # 16
N_CHUNKS = K_DIM // 128  # 4
N_STRIPS = 5
MEANINGS_PER_TYPE = 10
SCALE = 32.0  # support multiplier; needs 2^5 > N_BLOCKS
INIT = float(1 << 24)

_CACHE = {}
LAST_RESULTS = None  # BassKernelResults of the most recent device run
LAST_WALL_NS = None


def _build_bass(reps=1):
    import concourse.bacc as bacc
    import concourse.tile as tile
    from concourse import mybir

    nc = bacc.Bacc(
        "TRN2", target_bir_lowering=False, debug=False, enable_asserts=False
    )
    bf16 = mybir.dt.bfloat16
    f32 = mybir.dt.float32

    supT = nc.dram_tensor("supT", [K_DIM, S_PAD], bf16, kind="ExternalInput").ap()
    bfT = nc.dram_tensor("bfT", [K_DIM, N_QUERIES], bf16, kind="ExternalInput").ap()
    jrow = nc.dram_tensor("jrow", [128, BLOCK], f32, kind="ExternalInput").ap()
    out = nc.dram_tensor("out", [128, N_TILES], f32, kind="ExternalOutput").ap()

    QW = S_PAD // N_STRIPS  # 2560 columns (5 blocks) per resident support strip

    with tile.TileContext(nc) as tc:
        with (
            tc.tile_pool(name="sup", bufs=1) as sup_pool,
            tc.tile_pool(name="bq", bufs=1) as bq_pool,
            tc.tile_pool(name="ps", bufs=8, space="PSUM") as ps_pool,
            tc.tile_pool(name="run", bufs=1) as run_pool,
            tc.tile_pool(name="fin", bufs=1) as fin_pool,
        ):

            def body():
                sup_tiles = {}
                for c in range(N_CHUNKS):
                    for q in range(N_STRIPS):
                        t = sup_pool.tile(
                            [128, QW], bf16, tag=f"sup{c}_{q}", name=f"sup{c}_{q}"
                        )
                        nc.sync.dma_start(
                            t[:], supT[128 * c : 128 * (c + 1), QW * q : QW * (q + 1)]
                        )
                        sup_tiles[(c, q)] = t

                bq_tiles = []
                for c in range(N_CHUNKS):
                    t = bq_pool.tile(
                        [128, N_QUERIES], bf16, tag=f"bq{c}", name=f"bq{c}"
                    )
                    nc.sync.dma_start(t[:], bfT[128 * c : 128 * (c + 1), :])
                    bq_tiles.append(t)

                jrow_t = fin_pool.tile([128, BLOCK], f32, tag="jrow", name="jrow_t")
                nc.sync.dma_start(jrow_t[:], jrow[:])

                run_tiles = []
                for nt in range(N_TILES):
                    t = run_pool.tile(
                        [128, BLOCK], f32, tag=f"run{nt}", name=f"run{nt}"
                    )
                    nc.gpsimd.memset(t[:], INIT)
                    run_tiles.append(t)

                for b in range(N_BLOCKS):
                    q, rem = divmod(BLOCK * b, QW)
                    for nt in range(N_TILES):
                        ps = ps_pool.tile(
                            [128, BLOCK], f32, tag="ps", name=f"ps{b}_{nt}"
                        )
                        for c in range(N_CHUNKS):
                            nc.tensor.matmul(
                                ps[:],
                                bq_tiles[c][:, 128 * nt : 128 * (nt + 1)],
                                sup_tiles[(c, q)][:, rem : rem + BLOCK],
                                start=(c == 0),
                                stop=(c == N_CHUNKS - 1),
                            )
                        # run = min(psum + b, run): value+block-index, exact
                        nc.vector.scalar_tensor_tensor(
                            out=run_tiles[nt][:],
                            in0=ps[:],
                            scalar=float(b),
                            in1=run_tiles[nt][:],
                            op0=mybir.AluOpType.add,
                            op1=mybir.AluOpType.min,
                        )

                fin = fin_pool.tile([128, N_TILES], f32, tag="fin", name="fin")
                for nt in range(N_TILES):
                    key = fin_pool.tile([128, BLOCK], f32, tag="key", name=f"key{nt}")
                    # key = run*512 + j  ==  -2^14*match + s_local, exact
                    nc.vector.scalar_tensor_tensor(
                        out=key[:],
                        in0=run_tiles[nt][:],
                        scalar=float(BLOCK),
                        in1=jrow_t[:],
                        op0=mybir.AluOpType.mult,
                        op1=mybir.AluOpType.add,
                    )
                    nc.vector.tensor_reduce(
                        out=fin[:, nt : nt + 1],
                        in_=key[:],
                        axis=mybir.AxisListType.X,
                        op=mybir.AluOpType.min,
                    )
                nc.sync.dma_start(out[:], fin[:])

            if reps == 1:
                body()
            else:
                with tc.For_i(0, reps, 1):
                    body()

    nc.compile()
    return nc


def _get_nc(reps=1):
    key = ("nc", reps)
    if key not in _CACHE:
        _CACHE[key] = _build_bass(reps)
    return _CACHE[key]


def _make_timed_runner(nc, in_maps):
    """Replicates bass2jax.run_bass_via_pjrt's sharded call, but with
    device-resident inputs so repeated invocations time dispatch+execute
    only (no host->device transfer of the 100MB+ of inputs)."""
    import jax
    from jax.sharding import Mesh, NamedSharding, PartitionSpec

    from jax.experimental.shard_map import shard_map

    from concourse import bass2jax, mybir
    from concourse.bass2jax import _bass_exec_p, install_neuronx_cc_hook

    install_neuronx_cc_hook()
    partition_name = (
        nc.partition_id_tensor.name if nc.partition_id_tensor else None
    )
    in_names, out_names, out_avals, zero_outs = [], [], [], []
    for alloc in nc.m.functions[0].allocations:
        if not isinstance(alloc, mybir.MemoryLocationSet):
            continue
        name = alloc.memorylocations[0].name
        if alloc.kind == "ExternalInput":
            if name != partition_name:
                in_names.append(name)
        elif alloc.kind == "ExternalOutput":
            out_names.append(name)
            shape = tuple(alloc.tensor_shape)
            dtype = mybir.dt.np(alloc.dtype)
            out_avals.append(jax.core.ShapedArray(shape, dtype))
            zero_outs.append(np.zeros(shape, dtype))
    n_params = len(in_names)
    n_outs = len(out_avals)
    in_names_full = list(in_names) + out_names
    if partition_name is not None:
        in_names_full.append(partition_name)

    def _body(*args):
        operands = list(args)
        if partition_name is not None:
            operands.append(bass2jax.partition_id_tensor())
        return tuple(
            _bass_exec_p.bind(
                *operands,
                out_avals=tuple(out_avals),
                in_names=tuple(in_names_full),
                out_names=tuple(out_names),
                lowering_input_output_aliases=(),
                sim_require_finite=True,
                sim_require_nnan=True,
                nc=nc,
            )
        )

    devices = jax.devices()[:N_CORES]
    mesh = Mesh(np.asarray(devices), ("core",))
    in_specs = (PartitionSpec("core"),) * (n_params + n_outs)
    out_specs = (PartitionSpec("core"),) * len(out_names)
    donate = tuple(range(n_params, n_params + n_outs))
    sharded = jax.jit(
        shard_map(
            _body, mesh=mesh, in_specs=in_specs, out_specs=out_specs,
            check_rep=False,
        ),
        donate_argnums=donate,
        keep_unused=True,
    )
    sh = NamedSharding(mesh, PartitionSpec("core"))
    concat_in = [
        np.concatenate([np.asarray(in_maps[c][nm]) for c in range(N_CORES)], axis=0)
        for nm in in_names
    ]
    dev_in = [jax.device_put(a, sh) for a in concat_in]

    def call():
        zs = [
            jax.device_put(
                np.zeros((N_CORES * z.shape[0], *z.shape[1:]), z.dtype), sh
            )
            for z in zero_outs
        ]
        jax.block_until_ready(zs)
        t0 = time.perf_counter_ns()
        outs = sharded(*dev_in, *zs)
        jax.block_until_ready(outs)
        dt = time.perf_counter_ns() - t0
        return dt, outs

    return call


def measure_hw_exec_ns(in_maps, r1=25, r2=225, tries=8):
    """Per-iteration device time of the full kernel body, measured by
    differencing two in-NEFF repetition counts (cancels dispatch/RPC)."""
    times = {}
    for r in (r1, r2):
        call = _make_timed_runner(_get_nc(reps=r), in_maps)
        call()  # warmup/compile
        times[r] = min(call()[0] for _ in range(tries))
    return (times[r2] - times[r1]) / (r2 - r1), times


def _prep_in_maps(utts_np, support_np):
    bf = utts_np.astype(np.int64)[:, None, :] == np.arange(VOCAB, dtype=np.int64)[
        None, :, None
    ]
    bfT = bf.reshape(K_DIM, N_QUERIES).astype(ml_dtypes.bfloat16)
    jrow = np.ascontiguousarray(
        np.broadcast_to(np.arange(BLOCK, dtype=np.float32), (128, BLOCK))
    )

    in_maps = []
    for c in range(N_CORES):
        shard = support_np[c * S_SHARD : (c + 1) * S_SHARD]  # [12500, 512]
        supT_c = np.zeros((K_DIM, S_PAD), dtype=ml_dtypes.bfloat16)
        supT_c[:, :S_SHARD] = (shard.T * (-SCALE)).astype(ml_dtypes.bfloat16)
        in_maps.append({"supT": supT_c, "bfT": bfT, "jrow": jrow})
    return in_maps


def _one_hot_meanings(meanings_np, idx):
    meanings = np.asarray(meanings_np)[idx]  # [2048, T]
    n, t = meanings.shape
    out = np.zeros((n, t, MEANINGS_PER_TYPE), dtype=np.float32)
    out[np.arange(n)[:, None], np.arange(t)[None, :], meanings.astype(np.int64)] = 1.0
    return out


def _fallback_numpy(utts_np, support_np, meanings_np):
    """Exact reference semantics in fp32 numpy (for unexpected inputs)."""
    u = utts_np.astype(np.int64)
    m, n = u.shape
    bf = (u.T[:, :, None] == np.arange(VOCAB, dtype=np.int64)).astype(np.float32)
    bf = bf.reshape(n, m * VOCAB)
    sup = support_np.astype(np.float32)
    sup_sq = np.sum(sup * sup, axis=1)
    best_val = np.full(n, np.inf, dtype=np.float32)
    best_idx = np.zeros(n, dtype=np.int64)
    ch = 8192
    for s0 in range(0, sup.shape[0], ch):
        blk = sup[s0 : s0 + ch]
        d = sup_sq[s0 : s0 + ch][None, :] - 2.0 * (bf @ blk.T)
        i = np.argmin(d, axis=1)
        v = d[np.arange(n), i]
        upd = v < best_val  # strict: keeps first occurrence
        best_idx[upd] = s0 + i[upd]
        best_val[upd] = v[upd]
    return _one_hot_meanings(meanings_np, best_idx)


def _is_fast_path(utts_np, support_np, meanings_np):
    if utts_np.shape != (UTT_LEN, N_QUERIES):
        return False
    if support_np.shape != (S_FULL, K_DIM):
        return False
    if meanings_np.shape[0] != S_FULL:
        return False
    if utts_np.min() < 0 or utts_np.max() >= VOCAB:
        return False
    # exact encoding requires {0,1}-valued support with constant row norms
    if not np.all((support_np == 0.0) | (support_np == 1.0)):
        return False
    rs = support_np.sum(axis=1)
    if not np.all(rs == rs[0]):
        return False
    return True


def kernel(utts, support, meanings_t, _trace=False, **_trace_kwargs):
    global LAST_RESULTS, LAST_WALL_NS
    utts_np = np.asarray(utts)
    support_np = np.asarray(support, dtype=np.float32)
    meanings_np = np.asarray(meanings_t)

    if not _is_fast_path(utts_np, support_np, meanings_np):
        return _fallback_numpy(utts_np, support_np, meanings_np)

    from concourse.bass_utils import run_bass_kernel_spmd

    nc = _get_nc()
    in_maps = _prep_in_maps(utts_np, support_np)
    t0 = time.monotonic_ns()
    res = run_bass_kernel_spmd(
        nc, in_maps, list(range(N_CORES)), trace=_trace, **_trace_kwargs
    )
    LAST_WALL_NS = time.monotonic_ns() - t0
    LAST_RESULTS = res

    vals = np.stack(
        [np.asarray(r["out"], dtype=np.float32) for r in res.results]
    )  # [8, 128, 16]: [core, p, t] -> query 128*t + p
    keys = np.rint(vals.transpose(0, 2, 1).reshape(N_CORES, N_QUERIES)).astype(
        np.int64
    )
    s_local = np.mod(keys, 1 << 14)
    match = (s_local - keys) >> 14  # match_count per core winner
    s_global = s_local + (np.arange(N_CORES, dtype=np.int64) * S_SHARD)[:, None]
    # global winner: max match_count, then smallest global index
    host_key = -match * (1 << 40) + s_global
    win = np.argmin(host_key, axis=0)
    idx = s_global[win, np.arange(N_QUERIES)]
    return _one_hot_meanings(meanings_np, idx)


# revision 10
# speedup vs baseline: 1.6024x; 1.6024x over previous
"""Sharded KNN retrieval kernel for Trainium2 (8 NeuronCores).

Problem: 2048 one-hot-encoded query utterances vs 100k one-hot-encoded
support utterances; top-1 nearest neighbor by squared L2, first-index
tie-breaking; output = one-hot of the winner's meanings row.

Because both sides are one-hot, squared distance reduces to
    dist(n, s) = const - 2 * match_count(n, s),   match_count in [0, 16]
so argmin(dist) = argmax(match_count) with first-index tie-break. All
arithmetic is small integers, exact in bf16/fp32, so value+index are encoded
in the matmul output itself:

  support sharded 12500 rows/core (padded to 12800 = 25 blocks of 512).
  TensorE:  psum[n_tile, s_block] = bfT.T @ (-32 * supT)   (= -32*match_count)
  VectorE:  run[nt] = min(psum + block_idx, run[nt])       (one fused
            scalar_tensor_tensor per psum tile; -32*c + b is exact and
            orders by (match_count desc, block asc) since b < 32)
  Final:    key = run*512 + j  (j = within-block offset; equals the exact
            encoding -2^14*c + s_local), then min-reduce over j.
  Host:     per-core decode (c, s_local), global lexicographic min over
            cores by (match_count desc, global_index asc), gather meanings,
            one-hot. Everything is exact integer arithmetic in fp32.
"""

import sys
import time

import numpy as np

if "/opt/trn_rl_repo" not in sys.path:
    sys.path.insert(0, "/opt/trn_rl_repo")

import ml_dtypes

VOCAB = 32
UTT_LEN = 16
K_DIM = VOCAB * UTT_LEN  # 512
N_QUERIES = 2048
S_FULL = 100000
N_CORES = 8
S_SHARD = S_FULL // N_CORES  # 12500
S_PAD = 12800
BLOCK = 512
N_BLOCKS = S_PAD // BLOCK  # 25
N_TILES = N_QUERIES // 128  # 16
N_CHUNKS = K_DIM // 128  # 4
N_STRIPS = 5
MEANINGS_PER_TYPE = 10
SCALE = 32.0  # support multiplier; needs 2^5 > N_BLOCKS
INIT = float(1 << 24)
USE_FP8 = True  # fp8e4 DoubleRow matmuls (exact for {0,1}/{-32,0} values)

_CACHE = {}
LAST_RESULTS = None  # BassKernelResults of the most recent device run
LAST_WALL_NS = None


def _build_bass(reps=1, fp8=USE_FP8):
    import concourse.bacc as bacc
    import concourse.tile as tile
    from concourse import mybir

    nc = bacc.Bacc(
        "TRN2", target_bir_lowering=False, debug=False, enable_asserts=False
    )
    bf16 = mybir.dt.bfloat16
    fp8e4 = mybir.dt.float8e4
    f32 = mybir.dt.float32

    if fp8:
        # DoubleRow layout: K=512 split as 2 groups x (2 k-tiles x 128)
        supT = nc.dram_tensor(
            "supT", [2, 128, 2, S_PAD], fp8e4, kind="ExternalInput"
        ).ap()
        bfT = nc.dram_tensor(
            "bfT", [2, 128, 2, N_QUERIES], fp8e4, kind="ExternalInput"
        ).ap()
    else:
        supT = nc.dram_tensor("supT", [K_DIM, S_PAD], bf16, kind="ExternalInput").ap()
        bfT = nc.dram_tensor(
            "bfT", [K_DIM, N_QUERIES], bf16, kind="ExternalInput"
        ).ap()
    jrow = nc.dram_tensor("jrow", [128, BLOCK], f32, kind="ExternalInput").ap()
    out = nc.dram_tensor("out", [128, N_TILES], f32, kind="ExternalOutput").ap()

    QW = S_PAD // N_STRIPS  # 2560 columns (5 blocks) per resident support strip
    n_groups = 2 if fp8 else N_CHUNKS

    with tile.TileContext(nc) as tc:
        with (
            tc.tile_pool(name="sup", bufs=1) as sup_pool,
            tc.tile_pool(name="bq", bufs=1) as bq_pool,
            tc.tile_pool(name="ps", bufs=8, space="PSUM") as ps_pool,
            tc.tile_pool(name="run", bufs=1) as run_pool,
            tc.tile_pool(name="fin", bufs=1) as fin_pool,
        ):

            def body():
                sup_tiles = {}
                bq_tiles = []
                if fp8:
                    for c in range(n_groups):
                        for q in range(N_STRIPS):
                            t = sup_pool.tile(
                                [128, 2, QW], fp8e4,
                                tag=f"sup{c}_{q}", name=f"sup{c}_{q}",
                            )
                            nc.sync.dma_start(
                                t[:], supT[c, :, :, QW * q : QW * (q + 1)]
                            )
                            sup_tiles[(c, q)] = t
                    for c in range(n_groups):
                        t = bq_pool.tile(
                            [128, 2, N_QUERIES], fp8e4, tag=f"bq{c}", name=f"bq{c}"
                        )
                        nc.sync.dma_start(t[:], bfT[c])
                        bq_tiles.append(t)
                else:
                    for c in range(n_groups):
                        for q in range(N_STRIPS):
                            t = sup_pool.tile(
                                [128, QW], bf16, tag=f"sup{c}_{q}", name=f"sup{c}_{q}"
                            )
                            nc.sync.dma_start(
                                t[:],
                                supT[128 * c : 128 * (c + 1), QW * q : QW * (q + 1)],
                            )
                            sup_tiles[(c, q)] = t
                    for c in range(n_groups):
                        t = bq_pool.tile(
                            [128, N_QUERIES], bf16, tag=f"bq{c}", name=f"bq{c}"
                        )
                        nc.sync.dma_start(t[:], bfT[128 * c : 128 * (c + 1), :])
                        bq_tiles.append(t)

                jrow_t = fin_pool.tile([128, BLOCK], f32, tag="jrow", name="jrow_t")
                nc.sync.dma_start(jrow_t[:], jrow[:])

                run_tiles = []
                for nt in range(N_TILES):
                    t = run_pool.tile(
                        [128, BLOCK], f32, tag=f"run{nt}", name=f"run{nt}"
                    )
                    nc.gpsimd.memset(t[:], INIT)
                    run_tiles.append(t)

                for b in range(N_BLOCKS):
                    q, rem = divmod(BLOCK * b, QW)
                    for nt in range(N_TILES):
                        ps = ps_pool.tile(
                            [128, BLOCK], f32, tag="ps", name=f"ps{b}_{nt}"
                        )
                        for c in range(n_groups):
                            if fp8:
                                nc.tensor.matmul(
                                    ps[:],
                                    bq_tiles[c][:, :, 128 * nt : 128 * (nt + 1)],
                                    sup_tiles[(c, q)][:, :, rem : rem + BLOCK],
                                    start=(c == 0),
                                    stop=(c == n_groups - 1),
                                    perf_mode=mybir.MatmulPerfMode.DoubleRow,
                                )
                            else:
                                nc.tensor.matmul(
                                    ps[:],
                                    bq_tiles[c][:, 128 * nt : 128 * (nt + 1)],
                                    sup_tiles[(c, q)][:, rem : rem + BLOCK],
                                    start=(c == 0),
                                    stop=(c == n_groups - 1),
                                )
                        # run = min(psum + b, run): value+block-index, exact
                        nc.vector.scalar_tensor_tensor(
                            out=run_tiles[nt][:],
                            in0=ps[:],
                            scalar=float(b),
                            in1=run_tiles[nt][:],
                            op0=mybir.AluOpType.add,
                            op1=mybir.AluOpType.min,
                        )

                fin = fin_pool.tile([128, N_TILES], f32, tag="fin", name="fin")
                for nt in range(N_TILES):
                    key = fin_pool.tile([128, BLOCK], f32, tag="key", name=f"key{nt}")
                    # key = run*512 + j  ==  -2^14*match + s_local, exact
                    nc.vector.scalar_tensor_tensor(
                        out=key[:],
                        in0=run_tiles[nt][:],
                        scalar=float(BLOCK),
                        in1=jrow_t[:],
                        op0=mybir.AluOpType.mult,
                        op1=mybir.AluOpType.add,
                    )
                    nc.vector.tensor_reduce(
                        out=fin[:, nt : nt + 1],
                        in_=key[:],
                        axis=mybir.AxisListType.X,
                        op=mybir.AluOpType.min,
                    )
                nc.sync.dma_start(out[:], fin[:])

            if reps == 1:
                body()
            else:
                with tc.For_i(0, reps, 1):
                    body()

    nc.compile()
    return nc


def _get_nc(reps=1, fp8=None):
    if fp8 is None:
        fp8 = USE_FP8
    key = ("nc", reps, fp8)
    if key not in _CACHE:
        _CACHE[key] = _build_bass(reps, fp8)
    return _CACHE[key]


def _make_timed_runner(nc, in_maps):
    """Replicates bass2jax.run_bass_via_pjrt's sharded call, but with
    device-resident inputs so repeated invocations time dispatch+execute
    only (no host->device transfer of the 100MB+ of inputs)."""
    import jax
    from jax.sharding import Mesh, NamedSharding, PartitionSpec

    from jax.experimental.shard_map import shard_map

    from concourse import bass2jax, mybir
    from concourse.bass2jax import _bass_exec_p, install_neuronx_cc_hook

    install_neuronx_cc_hook()
    partition_name = (
        nc.partition_id_tensor.name if nc.partition_id_tensor else None
    )
    in_names, out_names, out_avals, zero_outs = [], [], [], []
    for alloc in nc.m.functions[0].allocations:
        if not isinstance(alloc, mybir.MemoryLocationSet):
            continue
        name = alloc.memorylocations[0].name
        if alloc.kind == "ExternalInput":
            if name != partition_name:
                in_names.append(name)
        elif alloc.kind == "ExternalOutput":
            out_names.append(name)
            shape = tuple(alloc.tensor_shape)
            dtype = mybir.dt.np(alloc.dtype)
            out_avals.append(jax.core.ShapedArray(shape, dtype))
            zero_outs.append(np.zeros(shape, dtype))
    n_params = len(in_names)
    n_outs = len(out_avals)
    in_names_full = list(in_names) + out_names
    if partition_name is not None:
        in_names_full.append(partition_name)

    def _body(*args):
        operands = list(args)
        if partition_name is not None:
            operands.append(bass2jax.partition_id_tensor())
        return tuple(
            _bass_exec_p.bind(
                *operands,
                out_avals=tuple(out_avals),
                in_names=tuple(in_names_full),
                out_names=tuple(out_names),
                lowering_input_output_aliases=(),
                sim_require_finite=True,
                sim_require_nnan=True,
                nc=nc,
            )
        )

    devices = jax.devices()[:N_CORES]
    mesh = Mesh(np.asarray(devices), ("core",))
    in_specs = (PartitionSpec("core"),) * (n_params + n_outs)
    out_specs = (PartitionSpec("core"),) * len(out_names)
    donate = tuple(range(n_params, n_params + n_outs))
    sharded = jax.jit(
        shard_map(
            _body, mesh=mesh, in_specs=in_specs, out_specs=out_specs,
            check_rep=False,
        ),
        donate_argnums=donate,
        keep_unused=True,
    )
    sh = NamedSharding(mesh, PartitionSpec("core"))
    concat_in = [
        np.concatenate([np.asarray(in_maps[c][nm]) for c in range(N_CORES)], axis=0)
        for nm in in_names
    ]
    dev_in = [jax.device_put(a, sh) for a in concat_in]

    def call():
        zs = [
            jax.device_put(
                np.zeros((N_CORES * z.shape[0], *z.shape[1:]), z.dtype), sh
            )
            for z in zero_outs
        ]
        jax.block_until_ready(zs)
        t0 = time.perf_counter_ns()
        outs = sharded(*dev_in, *zs)
        jax.block_until_ready(outs)
        dt = time.perf_counter_ns() - t0
        return dt, outs

    return call


def measure_hw_exec_ns(in_maps, r1=25, r2=225, tries=8):
    """Per-iteration device time of the full kernel body, measured by
    differencing two in-NEFF repetition counts (cancels dispatch/RPC)."""
    times = {}
    for r in (r1, r2):
        call = _make_timed_runner(_get_nc(reps=r), in_maps)
        call()  # warmup/compile
        times[r] = min(call()[0] for _ in range(tries))
    return (times[r2] - times[r1]) / (r2 - r1), times


def _dr_pack(mat_f32, dt):
    """[512, W] -> [2, 128, 2, W] DoubleRow k-tile packing: k = 256*g + 128*ko + ki."""
    w = mat_f32.shape[1]
    return np.ascontiguousarray(
        mat_f32.reshape(2, 2, 128, w).transpose(0, 2, 1, 3)
    ).astype(dt)


def _prep_in_maps(utts_np, support_np, fp8=None):
    if fp8 is None:
        fp8 = USE_FP8
    bf = utts_np.astype(np.int64)[:, None, :] == np.arange(VOCAB, dtype=np.int64)[
        None, :, None
    ]
    bfT = bf.reshape(K_DIM, N_QUERIES).astype(np.float32)
    jrow = np.ascontiguousarray(
        np.broadcast_to(np.arange(BLOCK, dtype=np.float32), (128, BLOCK))
    )
    if fp8:
        bfT_in = _dr_pack(bfT, ml_dtypes.float8_e4m3)
    else:
        bfT_in = bfT.astype(ml_dtypes.bfloat16)

    in_maps = []
    for c in range(N_CORES):
        shard = support_np[c * S_SHARD : (c + 1) * S_SHARD]  # [12500, 512]
        supT_c = np.zeros((K_DIM, S_PAD), dtype=np.float32)
        supT_c[:, :S_SHARD] = shard.T * (-SCALE)
        if fp8:
            supT_in = _dr_pack(supT_c, ml_dtypes.float8_e4m3)
        else:
            supT_in = supT_c.astype(ml_dtypes.bfloat16)
        in_maps.append({"supT": supT_in, "bfT": bfT_in, "jrow": jrow})
    return in_maps


def _one_hot_meanings(meanings_np, idx):
    meanings = np.asarray(meanings_np)[idx]  # [2048, T]
    n, t = meanings.shape
    out = np.zeros((n, t, MEANINGS_PER_TYPE), dtype=np.float32)
    out[np.arange(n)[:, None], np.arange(t)[None, :], meanings.astype(np.int64)] = 1.0
    return out


def _fallback_numpy(utts_np, support_np, meanings_np):
    """Exact reference semantics in fp32 numpy (for unexpected inputs)."""
    u = utts_np.astype(np.int64)
    m, n = u.shape
    bf = (u.T[:, :, None] == np.arange(VOCAB, dtype=np.int64)).astype(np.float32)
    bf = bf.reshape(n, m * VOCAB)
    sup = support_np.astype(np.float32)
    sup_sq = np.sum(sup * sup, axis=1)
    best_val = np.full(n, np.inf, dtype=np.float32)
    best_idx = np.zeros(n, dtype=np.int64)
    ch = 8192
    for s0 in range(0, sup.shape[0], ch):
        blk = sup[s0 : s0 + ch]
        d = sup_sq[s0 : s0 + ch][None, :] - 2.0 * (bf @ blk.T)
        i = np.argmin(d, axis=1)
        v = d[np.arange(n), i]
        upd = v < best_val  # strict: keeps first occurrence
        best_idx[upd] = s0 + i[upd]
        best_val[upd] = v[upd]
    return _one_hot_meanings(meanings_np, best_idx)


def _is_fast_path(utts_np, support_np, meanings_np):
    if utts_np.shape != (UTT_LEN, N_QUERIES):
        return False
    if support_np.shape != (S_FULL, K_DIM):
        return False
    if meanings_np.shape[0] != S_FULL:
        return False
    if utts_np.min() < 0 or utts_np.max() >= VOCAB:
        return False
    # exact encoding requires {0,1}-valued support with constant row norms
    if not np.all((support_np == 0.0) | (support_np == 1.0)):
        return False
    rs = support_np.sum(axis=1)
    if not np.all(rs == rs[0]):
        return False
    return True


def kernel(utts, support, meanings_t, _trace=False, **_trace_kwargs):
    global LAST_RESULTS, LAST_WALL_NS
    utts_np = np.asarray(utts)
    support_np = np.asarray(support, dtype=np.float32)
    meanings_np = np.asarray(meanings_t)

    if not _is_fast_path(utts_np, support_np, meanings_np):
        return _fallback_numpy(utts_np, support_np, meanings_np)

    from concourse.bass_utils import run_bass_kernel_spmd

    nc = _get_nc()
    in_maps = _prep_in_maps(utts_np, support_np)
    t0 = time.monotonic_ns()
    res = run_bass_kernel_spmd(
        nc, in_maps, list(range(N_CORES)), trace=_trace, **_trace_kwargs
    )
    LAST_WALL_NS = time.monotonic_ns() - t0
    LAST_RESULTS = res

    vals = np.stack(
        [np.asarray(r["out"], dtype=np.float32) for r in res.results]
    )  # [8, 128, 16]: [core, p, t] -> query 128*t + p
    keys = np.rint(vals.transpose(0, 2, 1).reshape(N_CORES, N_QUERIES)).astype(
        np.int64
    )
    s_local = np.mod(keys, 1 << 14)
    match = (s_local - keys) >> 14  # match_count per core winner
    s_global = s_local + (np.arange(N_CORES, dtype=np.int64) * S_SHARD)[:, None]
    # global winner: max match_count, then smallest global index
    host_key = -match * (1 << 40) + s_global
    win = np.argmin(host_key, axis=0)
    idx = s_global[win, np.arange(N_QUERIES)]
    return _one_hot_meanings(meanings_np, idx)
